# revision 8
# baseline (speedup 1.0000x reference)
"""CODI minibatch loss (segment_reduce) on 8 Trainium2 NeuronCores.

Math: for each label k with count c_k, mean m_k = sums_k / c_k,
  sse_k = sum_{i in k} ||z_i + eps - m_k||^2
        = S2_k - ||sums_k||^2 / c_k + c_k * C*H * eps^2        (exact algebra)
  loss  = sum_{k: c_k>0} sse_k / (c_k * C*H)

The ||sums_k||^2 correction is only ~0.12% of the loss, so the per-label
feature sums tolerate reduced precision; the squared-norm path accumulates
in fp32. z ships to the device as fp16 (halves HBM traffic; ~1e-6 rel
effect on the loss).

Device work per core (batch-sharded, 1024 samples each):
  - S2 path: per-sample squared norms, free-axis accumulate in fp32.
    Split across ACT (Square activation) and DVE (tensor_tensor_reduce).
  - sums path: one-hot matmul on the PE. z chunk [128 samples, 128 feats]
    is the stationary operand (fp16 -> fast weight load), one-hot
    [128 samples, 10] the moving operand; output [128 feats, 10]
    accumulates over the 8 sample-tiles in a single PSUM bank laid out
    as [128, 40*10] fp32.
Host: tiny K x CH reduction in float64.
"""

import numpy as np

NUM_LABELS = 10
B_FULL = 8192
C, H = 20, 256
CH = C * H  # 5120
N_CORES = 8
B_LOCAL = B_FULL // N_CORES  # 1024
N_BTILES = B_LOCAL // 128  # 8
N_FCHUNK = CH // 128  # 40
EPS = 1e-8

_CACHE = {}
LAST_RESULT = None  # BassKernelResults of the most recent run (for test harness)


def _build_nc():
    import concourse.bacc as bacc
    import concourse.mybir as mybir
    import concourse.tile as tile

    nc = bacc.Bacc("TRN2", target_bir_lowering=False)
    z_in = nc.dram_tensor("z", [B_LOCAL, CH], mybir.dt.float16, kind="ExternalInput")
    oh_in = nc.dram_tensor(
        "onehot", [128, N_BTILES * NUM_LABELS], mybir.dt.float16, kind="ExternalInput"
    )
    sums_out = nc.dram_tensor(
        "sums", [128, N_FCHUNK * NUM_LABELS], mybir.dt.float32, kind="ExternalOutput"
    )
    snorm_out = nc.dram_tensor(
        "snorm", [128, N_BTILES], mybir.dt.float32, kind="ExternalOutput"
    )

    with tile.TileContext(nc) as tc:
        with (
            tc.tile_pool(name="zp", bufs=1) as zp,
            tc.tile_pool(name="sqp", bufs=4) as sqp,
            tc.tile_pool(name="small", bufs=1) as small,
            tc.tile_pool(name="ps", bufs=1, space="PSUM") as psp,
        ):
            # Queue all z loads immediately; HWDGE drains them FIFO so tiles
            # arrive staggered while compute runs.
            z_tiles = []
            for b in range(N_BTILES):
                zt = zp.tile([128, CH], mybir.dt.float16, tag=f"z{b}")
                nc.sync.dma_start(zt[:], z_in[b * 128 : (b + 1) * 128, :])
                z_tiles.append(zt)

            oh_all = small.tile([128, N_BTILES * NUM_LABELS], mybir.dt.float16)
            nc.sync.dma_start(oh_all[:], oh_in[:])
            s_all = small.tile([128, N_BTILES], mybir.dt.float32)
            psum = psp.tile([128, N_FCHUNK * NUM_LABELS], mybir.dt.float32)

            # Warm the ACT Square table while the first z DMA is in flight.
            warm = small.tile([128, 1], mybir.dt.float32)
            warm_acc = small.tile([128, 1], mybir.dt.float32)
            nc.gpsimd.memset(warm[:], 0.0)
            nc.scalar.activation(
                warm[:],
                warm[:],
                mybir.ActivationFunctionType.Square,
                accum_out=warm_acc[:],
            )

            for b in range(N_BTILES):
                zt = z_tiles[b]

                sq = sqp.tile([128, CH], mybir.dt.float16, tag="sq")
                if b in (1, 3, 5):
                    # DVE square+reduce costs ~8.3us/tile (reduce runs 1x);
                    # ACT does ~3.7us/tile, so DVE only takes two early tiles.
                    nc.vector.tensor_mul(sq[:], zt[:], zt[:])
                    nc.vector.reduce_sum(
                        s_all[:, b : b + 1], sq[:], axis=mybir.AxisListType.X
                    )
                else:
                    nc.scalar.activation(
                        sq[:],
                        zt[:],
                        mybir.ActivationFunctionType.Square,
                        accum_out=s_all[:, b : b + 1],
                    )

                for f in range(N_FCHUNK):
                    # start=True clears has_written for the WHOLE bank, so it
                    # may only be set on the very first matmul touching this
                    # bank; later slices overwrite-on-first-touch via the
                    # per-element has_written bits.
                    nc.tensor.matmul(
                        psum[:, f * NUM_LABELS : (f + 1) * NUM_LABELS],
                        zt[:, f * 128 : (f + 1) * 128],
                        oh_all[:, b * NUM_LABELS : (b + 1) * NUM_LABELS],
                        start=(b == 0 and f == 0),
                        stop=(b == N_BTILES - 1 and f == N_FCHUNK - 1),
                        skip_group_check=True,
                    )

            out_sb = small.tile([128, N_FCHUNK * NUM_LABELS], mybir.dt.float32)
            nc.vector.tensor_copy(out_sb[:], psum[:])
            nc.sync.dma_start(sums_out[:], out_sb[:])
            nc.sync.dma_start(snorm_out[:], s_all[:])

    nc.compile()
    return nc


def _get_nc():
    if "nc" not in _CACHE:
        _CACHE["nc"] = _build_nc()
    return _CACHE["nc"]


def kernel(z, labels):
    global LAST_RESULT
    from concourse.bass_utils import run_bass_kernel_spmd

    z = np.asarray(z)
    labels = np.asarray(labels).astype(np.int64)
    assert z.shape == (B_FULL, C, H), z.shape
    z2 = np.nan_to_num(z.reshape(B_FULL, CH)).astype(np.float16)

    onehot = np.zeros((B_FULL, NUM_LABELS), np.float16)
    onehot[np.arange(B_FULL), labels] = 1.0

    in_maps = []
    for c in range(N_CORES):
        zl = z2[c * B_LOCAL : (c + 1) * B_LOCAL]
        oh = (
            onehot[c * B_LOCAL : (c + 1) * B_LOCAL]
            .reshape(N_BTILES, 128, NUM_LABELS)
            .transpose(1, 0, 2)
            .reshape(128, N_BTILES * NUM_LABELS)
        )
        in_maps.append(
            {
                "z": np.ascontiguousarray(zl),
                "onehot": np.ascontiguousarray(oh),
            }
        )

    nc = _get_nc()
    res = run_bass_kernel_spmd(nc, in_maps, core_ids=list(range(N_CORES)))
    LAST_RESULT = res

    # Host gather/unshard: K x CH reduction in float64.
    counts = np.bincount(labels, minlength=NUM_LABELS).astype(np.float64)
    sums = np.zeros((NUM_LABELS, CH), np.float64)
    S2 = np.zeros(NUM_LABELS, np.float64)
    for c in range(N_CORES):
        r = res.results[c]
        arr = np.asarray(r["sums"]).reshape(128, N_FCHUNK, NUM_LABELS)
        sums += arr.transpose(2, 1, 0).reshape(NUM_LABELS, CH)
        s_flat = np.asarray(r["snorm"]).T.reshape(-1).astype(np.float64)  # b-major
        lab_loc = labels[c * B_LOCAL : (c + 1) * B_LOCAL]
        S2 += np.bincount(lab_loc, weights=s_flat, minlength=NUM_LABELS)

    c_safe = np.maximum(counts, 1.0)
    sse = S2 - (sums * sums).sum(axis=1) / c_safe + counts * CH * (EPS * EPS)
    mse = sse / (c_safe * CH)
    loss = np.where(counts > 0, mse, 0.0).sum()
    return np.float32(loss)


# revision 9
# speedup vs baseline: 1.1127x; 1.1127x over previous
"""CODI minibatch loss (segment_reduce) on 8 Trainium2 NeuronCores.

Math: for each label k with count c_k, mean m_k = sums_k / c_k,
  sse_k = sum_{i in k} ||z_i + eps - m_k||^2
        = S2_k - ||sums_k||^2 / c_k + c_k * C*H * eps^2        (exact algebra)
  loss  = sum_{k: c_k>0} sse_k / (c_k * C*H)

The ||sums_k||^2 correction is only ~0.12% of the loss, so the per-label
feature sums tolerate reduced precision; the squared-norm path accumulates
in fp32. z ships to the device as fp16 (halves HBM traffic; ~1e-6 rel
effect on the loss).

Device work per core (batch-sharded, 1024 samples each):
  - S2 path: per-sample squared norms, free-axis accumulate in fp32.
    Split across ACT (Square activation) and DVE (tensor_tensor_reduce).
  - sums path: one-hot matmul on the PE. z chunk [128 samples, 128 feats]
    is the stationary operand (fp16 -> fast weight load), one-hot
    [128 samples, 10] the moving operand; output [128 feats, 10]
    accumulates over the 8 sample-tiles in a single PSUM bank laid out
    as [128, 40*10] fp32.
Host: tiny K x CH reduction in float64.
"""

import numpy as np

NUM_LABELS = 10
B_FULL = 8192
C, H = 20, 256
CH = C * H  # 5120
N_CORES = 8
B_LOCAL = B_FULL // N_CORES  # 1024
N_BTILES = B_LOCAL // 128  # 8
N_FCHUNK = CH // 128  # 40
EPS = 1e-8

_CACHE = {}
LAST_RESULT = None  # BassKernelResults of the most recent run (for test harness)


def _build_nc():
    import concourse.bacc as bacc
    import concourse.mybir as mybir
    import concourse.tile as tile

    nc = bacc.Bacc("TRN2", target_bir_lowering=False)
    z_in = nc.dram_tensor("z", [B_LOCAL, CH], mybir.dt.float16, kind="ExternalInput")
    oh_in = nc.dram_tensor(
        "onehot", [128, N_BTILES * NUM_LABELS], mybir.dt.float16, kind="ExternalInput"
    )
    sums_out = nc.dram_tensor(
        "sums", [128, N_FCHUNK * NUM_LABELS], mybir.dt.float32, kind="ExternalOutput"
    )
    snorm_out = nc.dram_tensor(
        "snorm", [128, N_BTILES], mybir.dt.float32, kind="ExternalOutput"
    )

    with tile.TileContext(nc) as tc:
        with (
            tc.tile_pool(name="zp", bufs=1) as zp,
            tc.tile_pool(name="sqp", bufs=4) as sqp,
            tc.tile_pool(name="small", bufs=1) as small,
            tc.tile_pool(name="ps", bufs=1, space="PSUM") as psp,
        ):
            # One-hot first (tiny, and every matmul needs it), then queue all
            # z loads immediately; HWDGE drains them FIFO so tiles arrive
            # staggered while compute runs.
            oh_all = small.tile([128, N_BTILES * NUM_LABELS], mybir.dt.float16)
            nc.sync.dma_start(oh_all[:], oh_in[:])
            z_tiles = []
            for b in range(N_BTILES):
                zt = zp.tile([128, CH], mybir.dt.float16, tag=f"z{b}")
                nc.sync.dma_start(zt[:], z_in[b * 128 : (b + 1) * 128, :])
                z_tiles.append(zt)
            s_all = small.tile([128, N_BTILES], mybir.dt.float32)
            psum = psp.tile([128, N_FCHUNK * NUM_LABELS], mybir.dt.float32)

            # Warm the ACT Square table while the first z DMA is in flight.
            warm = small.tile([128, 1], mybir.dt.float32)
            warm_acc = small.tile([128, 1], mybir.dt.float32)
            nc.gpsimd.memset(warm[:], 0.0)
            nc.scalar.activation(
                warm[:],
                warm[:],
                mybir.ActivationFunctionType.Square,
                accum_out=warm_acc[:],
            )

            for b in range(N_BTILES):
                zt = z_tiles[b]

                sq = sqp.tile([128, CH], mybir.dt.float16, tag="sq")
                if b in (1, 3, 5):
                    # DVE square+reduce costs ~8.3us/tile (reduce runs 1x);
                    # ACT does ~3.7us/tile, so DVE only takes two early tiles.
                    nc.vector.tensor_mul(sq[:], zt[:], zt[:])
                    nc.vector.reduce_sum(
                        s_all[:, b : b + 1], sq[:], axis=mybir.AxisListType.X
                    )
                else:
                    nc.scalar.activation(
                        sq[:],
                        zt[:],
                        mybir.ActivationFunctionType.Square,
                        accum_out=s_all[:, b : b + 1],
                    )

                for f in range(N_FCHUNK):
                    # start=True clears has_written for the WHOLE bank, so it
                    # may only be set on the very first matmul touching this
                    # bank; later slices overwrite-on-first-touch via the
                    # per-element has_written bits.
                    nc.tensor.matmul(
                        psum[:, f * NUM_LABELS : (f + 1) * NUM_LABELS],
                        zt[:, f * 128 : (f + 1) * 128],
                        oh_all[:, b * NUM_LABELS : (b + 1) * NUM_LABELS],
                        start=(b == 0 and f == 0),
                        stop=(b == N_BTILES - 1 and f == N_FCHUNK - 1),
                        skip_group_check=True,
                    )

            out_sb = small.tile([128, N_FCHUNK * NUM_LABELS], mybir.dt.float32)
            nc.vector.tensor_copy(out_sb[:], psum[:])
            nc.sync.dma_start(sums_out[:], out_sb[:])
            nc.sync.dma_start(snorm_out[:], s_all[:])

    nc.compile()
    return nc


def _get_nc():
    if "nc" not in _CACHE:
        _CACHE["nc"] = _build_nc()
    return _CACHE["nc"]


def kernel(z, labels):
    global LAST_RESULT
    from concourse.bass_utils import run_bass_kernel_spmd

    z = np.asarray(z)
    labels = np.asarray(labels).astype(np.int64)
    assert z.shape == (B_FULL, C, H), z.shape
    z2 = np.nan_to_num(z.reshape(B_FULL, CH)).astype(np.float16)

    onehot = np.zeros((B_FULL, NUM_LABELS), np.float16)
    onehot[np.arange(B_FULL), labels] = 1.0

    in_maps = []
    for c in range(N_CORES):
        zl = z2[c * B_LOCAL : (c + 1) * B_LOCAL]
        oh = (
            onehot[c * B_LOCAL : (c + 1) * B_LOCAL]
            .reshape(N_BTILES, 128, NUM_LABELS)
            .transpose(1, 0, 2)
            .reshape(128, N_BTILES * NUM_LABELS)
        )
        in_maps.append(
            {
                "z": np.ascontiguousarray(zl),
                "onehot": np.ascontiguousarray(oh),
            }
        )

    nc = _get_nc()
    res = run_bass_kernel_spmd(nc, in_maps, core_ids=list(range(N_CORES)))
    LAST_RESULT = res

    # Host gather/unshard: K x CH reduction in float64.
    counts = np.bincount(labels, minlength=NUM_LABELS).astype(np.float64)
    sums = np.zeros((NUM_LABELS, CH), np.float64)
    S2 = np.zeros(NUM_LABELS, np.float64)
    for c in range(N_CORES):
        r = res.results[c]
        arr = np.asarray(r["sums"]).reshape(128, N_FCHUNK, NUM_LABELS)
        sums += arr.transpose(2, 1, 0).reshape(NUM_LABELS, CH)
        s_flat = np.asarray(r["snorm"]).T.reshape(-1).astype(np.float64)  # b-major
        lab_loc = labels[c * B_LOCAL : (c + 1) * B_LOCAL]
        S2 += np.bincount(lab_loc, weights=s_flat, minlength=NUM_LABELS)

    c_safe = np.maximum(counts, 1.0)
    sse = S2 - (sums * sums).sum(axis=1) / c_safe + counts * CH * (EPS * EPS)
    mse = sse / (c_safe * CH)
    loss = np.where(counts > 0, mse, 0.0).sum()
    return np.float32(loss)


# revision 13
# speedup vs baseline: 1.2361x; 1.1109x over previous
"""CODI minibatch loss (segment_reduce) on 8 Trainium2 NeuronCores.

Math: for each label k with count c_k, mean m_k = sums_k / c_k,
  sse_k = sum_{i in k} ||z_i + eps - m_k||^2
        = S2_k - ||sums_k||^2 / c_k + c_k * C*H * eps^2        (exact algebra)
  loss  = sum_{k: c_k>0} sse_k / (c_k * C*H)

The ||sums_k||^2 correction is only ~0.12% of the loss, so the per-label
feature sums tolerate reduced precision; the squared-norm path accumulates
in fp32. z ships to the device as fp16 (halves HBM traffic; ~1e-6 rel
effect on the loss).

Device work per core (batch-sharded, 1024 samples each):
  - S2 path: per-sample squared norms, free-axis accumulate in fp32.
    Split across ACT (Square activation) and DVE (tensor_tensor_reduce).
  - sums path: one-hot matmul on the PE. z chunk [128 samples, 128 feats]
    is the stationary operand (fp16 -> fast weight load), one-hot
    [128 samples, 10] the moving operand; output [128 feats, 10]
    accumulates over the 8 sample-tiles in a single PSUM bank laid out
    as [128, 40*10] fp32.
Host: tiny K x CH reduction in float64.
"""

import numpy as np

NUM_LABELS = 10
B_FULL = 8192
C, H = 20, 256
CH = C * H  # 5120
N_CORES = 8
B_LOCAL = B_FULL // N_CORES  # 1024
N_BTILES = B_LOCAL // 128  # 8
N_FCHUNK = CH // 128  # 40
EPS = 1e-8

_CACHE = {}
LAST_RESULT = None  # BassKernelResults of the most recent run (for test harness)


def _build_nc():
    import concourse.bacc as bacc
    import concourse.mybir as mybir
    import concourse.tile as tile

    nc = bacc.Bacc("TRN2", target_bir_lowering=False)
    z_in = nc.dram_tensor("z", [B_LOCAL, CH], mybir.dt.float16, kind="ExternalInput")
    oh_in = nc.dram_tensor(
        "onehot", [128, N_BTILES * NUM_LABELS], mybir.dt.float16, kind="ExternalInput"
    )
    sums_out = nc.dram_tensor(
        "sums", [128, N_FCHUNK * NUM_LABELS], mybir.dt.float32, kind="ExternalOutput"
    )
    snorm_out = nc.dram_tensor(
        "snorm", [128, 2 * N_BTILES], mybir.dt.float32, kind="ExternalOutput"
    )

    with tile.TileContext(nc) as tc:
        with (
            tc.tile_pool(name="zp", bufs=1) as zp,
            tc.tile_pool(name="sqp", bufs=4) as sqp,
            tc.tile_pool(name="small", bufs=1) as small,
            tc.tile_pool(name="ps", bufs=1, space="PSUM") as psp,
        ):
            # One-hot on the scalar HWDGE ring (parallel with z0); z loads all
            # queued immediately on the sync ring, drained FIFO so tiles
            # arrive staggered while compute runs.
            oh_all = small.tile([128, N_BTILES * NUM_LABELS], mybir.dt.float16)
            nc.scalar.dma_start(oh_all[:], oh_in[:])
            z_tiles = []
            for b in range(N_BTILES):
                zt = zp.tile([128, CH], mybir.dt.float16, tag=f"z{b}")
                nc.sync.dma_start(zt[:], z_in[b * 128 : (b + 1) * 128, :])
                z_tiles.append(zt)
            # Per-sample sq-norm partials: column 2b = ACT part, 2b+1 = DVE
            # part; host adds them.
            s_all = small.tile([128, 2 * N_BTILES], mybir.dt.float32)
            psum = psp.tile([128, N_FCHUNK * NUM_LABELS], mybir.dt.float32)

            # Warm the ACT Square table while the first z DMA is in flight.
            warm = small.tile([128, 1], mybir.dt.float32)
            warm_acc = small.tile([128, 1], mybir.dt.float32)
            nc.gpsimd.memset(warm[:], 0.0)
            nc.scalar.activation(
                warm[:],
                warm[:],
                mybir.ActivationFunctionType.Square,
                accum_out=warm_acc[:],
            )

            # Column split sized so ACT (0.89 ns/col, fused square+accum) and
            # DVE (1.62 ns/col, mul at 2x + reduce at 1x) finish each tile
            # together, under the ~3.7us inter-arrival of z tiles.
            ACT_COLS = 3328
            for b in range(N_BTILES):
                zt = z_tiles[b]

                sqa = sqp.tile([128, ACT_COLS], mybir.dt.float16, tag="sqa")
                nc.scalar.activation(
                    sqa[:],
                    zt[:, :ACT_COLS],
                    mybir.ActivationFunctionType.Square,
                    accum_out=s_all[:, 2 * b : 2 * b + 1],
                )
                sqv = sqp.tile([128, CH - ACT_COLS], mybir.dt.float16, tag="sqv")
                nc.vector.tensor_mul(sqv[:], zt[:, ACT_COLS:], zt[:, ACT_COLS:])
                nc.vector.reduce_sum(
                    s_all[:, 2 * b + 1 : 2 * b + 2], sqv[:], axis=mybir.AxisListType.X
                )

                for f in range(N_FCHUNK):
                    # start=True clears has_written for the WHOLE bank, so it
                    # may only be set on the very first matmul touching this
                    # bank; later slices overwrite-on-first-touch via the
                    # per-element has_written bits.
                    nc.tensor.matmul(
                        psum[:, f * NUM_LABELS : (f + 1) * NUM_LABELS],
                        zt[:, f * 128 : (f + 1) * 128],
                        oh_all[:, b * NUM_LABELS : (b + 1) * NUM_LABELS],
                        start=(b == 0 and f == 0),
                        stop=(b == N_BTILES - 1 and f == N_FCHUNK - 1),
                        skip_group_check=True,
                    )

            out_sb = small.tile([128, N_FCHUNK * NUM_LABELS], mybir.dt.float32)
            nc.vector.tensor_copy(out_sb[:], psum[:])
            nc.sync.dma_start(sums_out[:], out_sb[:])
            nc.sync.dma_start(snorm_out[:], s_all[:])

    nc.compile()
    return nc


def _get_nc():
    if "nc" not in _CACHE:
        _CACHE["nc"] = _build_nc()
    return _CACHE["nc"]


def kernel(z, labels):
    global LAST_RESULT
    from concourse.bass_utils import run_bass_kernel_spmd

    z = np.asarray(z)
    labels = np.asarray(labels).astype(np.int64)
    assert z.shape == (B_FULL, C, H), z.shape
    z2 = np.nan_to_num(z.reshape(B_FULL, CH)).astype(np.float16)

    onehot = np.zeros((B_FULL, NUM_LABELS), np.float16)
    onehot[np.arange(B_FULL), labels] = 1.0

    in_maps = []
    for c in range(N_CORES):
        zl = z2[c * B_LOCAL : (c + 1) * B_LOCAL]
        oh = (
            onehot[c * B_LOCAL : (c + 1) * B_LOCAL]
            .reshape(N_BTILES, 128, NUM_LABELS)
            .transpose(1, 0, 2)
            .reshape(128, N_BTILES * NUM_LABELS)
        )
        in_maps.append(
            {
                "z": np.ascontiguousarray(zl),
                "onehot": np.ascontiguousarray(oh),
            }
        )

    nc = _get_nc()
    res = run_bass_kernel_spmd(nc, in_maps, core_ids=list(range(N_CORES)))
    LAST_RESULT = res

    # Host gather/unshard: K x CH reduction in float64.
    counts = np.bincount(labels, minlength=NUM_LABELS).astype(np.float64)
    sums = np.zeros((NUM_LABELS, CH), np.float64)
    S2 = np.zeros(NUM_LABELS, np.float64)
    for c in range(N_CORES):
        r = res.results[c]
        arr = np.asarray(r["sums"]).reshape(128, N_FCHUNK, NUM_LABELS)
        sums += arr.transpose(2, 1, 0).reshape(NUM_LABELS, CH)
        sn = np.asarray(r["snorm"]).astype(np.float64)  # [128, 2*N_BTILES]
        s_pb = sn[:, 0::2] + sn[:, 1::2]  # [128, N_BTILES] ACT part + DVE part
        s_flat = s_pb.T.reshape(-1)  # b-major
        lab_loc = labels[c * B_LOCAL : (c + 1) * B_LOCAL]
        S2 += np.bincount(lab_loc, weights=s_flat, minlength=NUM_LABELS)

    c_safe = np.maximum(counts, 1.0)
    sse = S2 - (sums * sums).sum(axis=1) / c_safe + counts * CH * (EPS * EPS)
    mse = sse / (c_safe * CH)
    loss = np.where(counts > 0, mse, 0.0).sum()
    return np.float32(loss)


# revision 17
# speedup vs baseline: 1.2760x; 1.0324x over previous
"""CODI minibatch loss (segment_reduce) on 8 Trainium2 NeuronCores.

Math: for each label k with count c_k, mean m_k = sums_k / c_k,
  sse_k = sum_{i in k} ||z_i + eps - m_k||^2
        = S2_k - ||sums_k||^2 / c_k + c_k * C*H * eps^2        (exact algebra)
  loss  = sum_{k: c_k>0} sse_k / (c_k * C*H)

The ||sums_k||^2 correction is only ~0.12% of the loss, so the per-label
feature sums tolerate reduced precision; the squared-norm path accumulates
in fp32. z ships to the device as fp16 (halves HBM traffic; ~1e-6 rel
effect on the loss).

Device work per core (batch-sharded, 1024 samples each):
  - S2 path: per-sample squared norms, free-axis accumulate in fp32.
    Split across ACT (Square activation) and DVE (tensor_tensor_reduce).
  - sums path: one-hot matmul on the PE. z chunk [128 samples, 128 feats]
    is the stationary operand (fp16 -> fast weight load), one-hot
    [128 samples, 10] the moving operand; output [128 feats, 10]
    accumulates over the 8 sample-tiles in a single PSUM bank laid out
    as [128, 40*10] fp32.
Host: tiny K x CH reduction in float64.
"""

import numpy as np

NUM_LABELS = 10
B_FULL = 8192
C, H = 20, 256
CH = C * H  # 5120
N_CORES = 8
B_LOCAL = B_FULL // N_CORES  # 1024
N_BTILES = B_LOCAL // 128  # 8
N_FCHUNK = CH // 128  # 40
EPS = 1e-8

_CACHE = {}
LAST_RESULT = None  # BassKernelResults of the most recent run (for test harness)


def _build_nc():
    import concourse.bacc as bacc
    import concourse.mybir as mybir
    import concourse.tile as tile

    nc = bacc.Bacc("TRN2", target_bir_lowering=False)
    z_in = nc.dram_tensor("z", [B_LOCAL, CH], mybir.dt.float16, kind="ExternalInput")
    oh_in = nc.dram_tensor(
        "onehot", [128, N_BTILES * NUM_LABELS], mybir.dt.float16, kind="ExternalInput"
    )
    sums_out = nc.dram_tensor(
        "sums", [128, N_FCHUNK * NUM_LABELS], mybir.dt.float32, kind="ExternalOutput"
    )
    snorm_out = nc.dram_tensor(
        "snorm", [128, 2 * N_BTILES + 2], mybir.dt.float32, kind="ExternalOutput"
    )

    with tile.TileContext(nc) as tc:
        with (
            tc.tile_pool(name="zp", bufs=1) as zp,
            tc.tile_pool(name="sqp", bufs=4) as sqp,
            tc.tile_pool(name="small", bufs=1) as small,
            tc.tile_pool(name="ps", bufs=1, space="PSUM") as psp,
        ):
            # One-hot on the scalar HWDGE ring (parallel with z0); z loads all
            # queued immediately on the sync ring, drained FIFO so tiles
            # arrive staggered while compute runs.
            oh_all = small.tile([128, N_BTILES * NUM_LABELS], mybir.dt.float16)
            nc.scalar.dma_start(oh_all[:], oh_in[:])
            z_tiles = []
            for b in range(N_BTILES - 1):
                zt = zp.tile([128, CH], mybir.dt.float16, tag=f"z{b}")
                nc.sync.dma_start(zt[:], z_in[b * 128 : (b + 1) * 128, :])
                z_tiles.append(zt)
            # Last tile split into two half-column tiles so its compute and
            # matmuls overlap the second half's transfer (shorter drain after
            # the final byte lands).
            CHH = CH // 2
            b7 = N_BTILES - 1
            z7a = zp.tile([128, CHH], mybir.dt.float16, tag="z7a")
            nc.sync.dma_start(z7a[:], z_in[b7 * 128 :, :CHH])
            z7b = zp.tile([128, CHH], mybir.dt.float16, tag="z7b")
            nc.sync.dma_start(z7b[:], z_in[b7 * 128 :, CHH:])
            # Per-sample sq-norm partials, summed on the host: columns 2b and
            # 2b+1 are the ACT/DVE parts of tile b (b<7); columns 14..17 are
            # the four partials of the split last tile.
            s_all = small.tile([128, 2 * N_BTILES + 2], mybir.dt.float32)
            psum = psp.tile([128, N_FCHUNK * NUM_LABELS], mybir.dt.float32)

            # Warm the ACT Square table while the first z DMA is in flight.
            warm = small.tile([128, 1], mybir.dt.float32)
            warm_acc = small.tile([128, 1], mybir.dt.float32)
            nc.gpsimd.memset(warm[:], 0.0)
            nc.scalar.activation(
                warm[:],
                warm[:],
                mybir.ActivationFunctionType.Square,
                accum_out=warm_acc[:],
            )

            # Column split sized so ACT (0.89 ns/col, fused square+accum) and
            # DVE (1.62 ns/col, mul at 2x + reduce at 1x) finish each tile
            # together, under the ~3.7us inter-arrival of z tiles.
            ACT_FRAC_NUM, ACT_FRAC_DEN = 13, 20  # ~0.65

            def squares(zt, ncols, col_a, col_v):
                act_cols = (ncols * ACT_FRAC_NUM // ACT_FRAC_DEN) // 128 * 128
                sqa = sqp.tile([128, act_cols], mybir.dt.float16, tag="sqa")
                nc.scalar.activation(
                    sqa[:],
                    zt[:, :act_cols],
                    mybir.ActivationFunctionType.Square,
                    accum_out=s_all[:, col_a : col_a + 1],
                )
                sqv = sqp.tile([128, ncols - act_cols], mybir.dt.float16, tag="sqv")
                nc.vector.tensor_mul(sqv[:], zt[:, act_cols:], zt[:, act_cols:])
                nc.vector.reduce_sum(
                    s_all[:, col_v : col_v + 1], sqv[:], axis=mybir.AxisListType.X
                )

            def mm(zt, f_local, b, f_global):
                # start=True clears has_written for the WHOLE bank, so it may
                # only be set on the very first matmul touching this bank;
                # later slices overwrite-on-first-touch via the per-element
                # has_written bits.
                nc.tensor.matmul(
                    psum[:, f_global * NUM_LABELS : (f_global + 1) * NUM_LABELS],
                    zt[:, f_local * 128 : (f_local + 1) * 128],
                    oh_all[:, b * NUM_LABELS : (b + 1) * NUM_LABELS],
                    start=(b == 0 and f_global == 0),
                    stop=(b == N_BTILES - 1 and f_global == N_FCHUNK - 1),
                    skip_group_check=True,
                )

            for b in range(N_BTILES - 1):
                zt = z_tiles[b]
                squares(zt, CH, 2 * b, 2 * b + 1)
                for f in range(N_FCHUNK):
                    mm(zt, f, b, f)

            # Split last tile: half A fully processed while half B transfers.
            squares(z7a, CHH, 14, 15)
            for f in range(N_FCHUNK // 2):
                mm(z7a, f, b7, f)
            squares(z7b, CHH, 16, 17)
            for f in range(N_FCHUNK // 2):
                mm(z7b, f, b7, f + N_FCHUNK // 2)

            out_sb = small.tile([128, N_FCHUNK * NUM_LABELS], mybir.dt.float32)
            nc.vector.tensor_copy(out_sb[:], psum[:])
            nc.sync.dma_start(sums_out[:], out_sb[:])
            # snorm on the scalar ring: issues right after the last
            # accumulator read, in parallel with the sums store.
            nc.scalar.dma_start(snorm_out[:], s_all[:])

    nc.compile()
    return nc


def _get_nc():
    if "nc" not in _CACHE:
        _CACHE["nc"] = _build_nc()
    return _CACHE["nc"]


def kernel(z, labels):
    global LAST_RESULT
    from concourse.bass_utils import run_bass_kernel_spmd

    z = np.asarray(z)
    labels = np.asarray(labels).astype(np.int64)
    assert z.shape == (B_FULL, C, H), z.shape
    z2 = np.nan_to_num(z.reshape(B_FULL, CH)).astype(np.float16)

    onehot = np.zeros((B_FULL, NUM_LABELS), np.float16)
    onehot[np.arange(B_FULL), labels] = 1.0

    in_maps = []
    for c in range(N_CORES):
        zl = z2[c * B_LOCAL : (c + 1) * B_LOCAL]
        oh = (
            onehot[c * B_LOCAL : (c + 1) * B_LOCAL]
            .reshape(N_BTILES, 128, NUM_LABELS)
            .transpose(1, 0, 2)
            .reshape(128, N_BTILES * NUM_LABELS)
        )
        in_maps.append(
            {
                "z": np.ascontiguousarray(zl),
                "onehot": np.ascontiguousarray(oh),
            }
        )

    nc = _get_nc()
    res = run_bass_kernel_spmd(nc, in_maps, core_ids=list(range(N_CORES)))
    LAST_RESULT = res

    # Host gather/unshard: K x CH reduction in float64.
    counts = np.bincount(labels, minlength=NUM_LABELS).astype(np.float64)
    sums = np.zeros((NUM_LABELS, CH), np.float64)
    S2 = np.zeros(NUM_LABELS, np.float64)
    for c in range(N_CORES):
        r = res.results[c]
        arr = np.asarray(r["sums"]).reshape(128, N_FCHUNK, NUM_LABELS)
        sums += arr.transpose(2, 1, 0).reshape(NUM_LABELS, CH)
        sn = np.asarray(r["snorm"]).astype(np.float64)  # [128, 18]
        s_pb = np.empty((128, N_BTILES))
        s_pb[:, : N_BTILES - 1] = (
            sn[:, 0 : 2 * (N_BTILES - 1) : 2] + sn[:, 1 : 2 * (N_BTILES - 1) : 2]
        )
        s_pb[:, N_BTILES - 1] = sn[:, 14:18].sum(axis=1)
        s_flat = s_pb.T.reshape(-1)  # b-major
        lab_loc = labels[c * B_LOCAL : (c + 1) * B_LOCAL]
        S2 += np.bincount(lab_loc, weights=s_flat, minlength=NUM_LABELS)

    c_safe = np.maximum(counts, 1.0)
    sse = S2 - (sums * sums).sum(axis=1) / c_safe + counts * CH * (EPS * EPS)
    mse = sse / (c_safe * CH)
    loss = np.where(counts > 0, mse, 0.0).sum()
    return np.float32(loss)


# revision 19
# speedup vs baseline: 1.2848x; 1.0068x over previous
"""CODI minibatch loss (segment_reduce) on 8 Trainium2 NeuronCores.

Math: for each label k with count c_k, mean m_k = sums_k / c_k,
  sse_k = sum_{i in k} ||z_i + eps - m_k||^2
        = S2_k - ||sums_k||^2 / c_k + c_k * C*H * eps^2        (exact algebra)
  loss  = sum_{k: c_k>0} sse_k / (c_k * C*H)

The ||sums_k||^2 correction is only ~0.12% of the loss, so the per-label
feature sums tolerate reduced precision; the squared-norm path accumulates
in fp32. z ships to the device as fp16 (halves HBM traffic; ~1e-6 rel
effect on the loss).

Device work per core (batch-sharded, 1024 samples each):
  - S2 path: per-sample squared norms, free-axis accumulate in fp32.
    Split across ACT (Square activation) and DVE (tensor_tensor_reduce).
  - sums path: one-hot matmul on the PE. z chunk [128 samples, 128 feats]
    is the stationary operand (fp16 -> fast weight load), one-hot
    [128 samples, 10] the moving operand; output [128 feats, 10]
    accumulates over the 8 sample-tiles in a single PSUM bank laid out
    as [128, 40*10] fp32.
Host: tiny K x CH reduction in float64.
"""

import numpy as np

NUM_LABELS = 10
B_FULL = 8192
C, H = 20, 256
CH = C * H  # 5120
N_CORES = 8
B_LOCAL = B_FULL // N_CORES  # 1024
N_BTILES = B_LOCAL // 128  # 8
N_FCHUNK = CH // 128  # 40
EPS = 1e-8

_CACHE = {}
LAST_RESULT = None  # BassKernelResults of the most recent run (for test harness)


def _build_nc():
    import concourse.bacc as bacc
    import concourse.mybir as mybir
    import concourse.tile as tile

    nc = bacc.Bacc("TRN2", target_bir_lowering=False)
    z_in = nc.dram_tensor("z", [B_LOCAL, CH], mybir.dt.float16, kind="ExternalInput")
    oh_in = nc.dram_tensor(
        "onehot", [128, N_BTILES * NUM_LABELS], mybir.dt.float16, kind="ExternalInput"
    )
    sums_out = nc.dram_tensor(
        "sums", [128, N_FCHUNK * NUM_LABELS], mybir.dt.float32, kind="ExternalOutput"
    )
    snorm_out = nc.dram_tensor(
        "snorm", [128, 2 * N_BTILES + 2], mybir.dt.float32, kind="ExternalOutput"
    )

    with tile.TileContext(nc) as tc:
        with (
            tc.tile_pool(name="zp", bufs=1) as zp,
            tc.tile_pool(name="sqp", bufs=4) as sqp,
            tc.tile_pool(name="small", bufs=1) as small,
            tc.tile_pool(name="ps", bufs=1, space="PSUM") as psp,
        ):
            # One-hot on the scalar HWDGE ring (parallel with z0); z loads all
            # queued immediately on the sync ring, drained FIFO so tiles
            # arrive staggered while compute runs.
            oh_all = small.tile([128, N_BTILES * NUM_LABELS], mybir.dt.float16)
            nc.scalar.dma_start(oh_all[:], oh_in[:])
            z_tiles = []
            for b in range(N_BTILES - 1):
                zt = zp.tile([128, CH], mybir.dt.float16, tag=f"z{b}")
                nc.sync.dma_start(zt[:], z_in[b * 128 : (b + 1) * 128, :])
                z_tiles.append(zt)
            # Last tile split into two half-column tiles so its compute and
            # matmuls overlap the second half's transfer (shorter drain after
            # the final byte lands).
            CHH = CH // 2
            b7 = N_BTILES - 1
            z7a = zp.tile([128, CHH], mybir.dt.float16, tag="z7a")
            nc.sync.dma_start(z7a[:], z_in[b7 * 128 :, :CHH])
            z7b = zp.tile([128, CHH], mybir.dt.float16, tag="z7b")
            nc.sync.dma_start(z7b[:], z_in[b7 * 128 :, CHH:])
            # Per-sample sq-norm partials, summed on the host: columns 2b and
            # 2b+1 are the ACT/DVE parts of tile b (b<7); columns 14..17 are
            # the four partials of the split last tile.
            s_all = small.tile([128, 2 * N_BTILES + 2], mybir.dt.float32)
            psum = psp.tile([128, N_FCHUNK * NUM_LABELS], mybir.dt.float32)

            # Column split sized so ACT (0.89 ns/col, fused square+accum) and
            # DVE (1.62 ns/col, mul at 2x + reduce at 1x) finish each tile
            # together, under the ~3.7us inter-arrival of z tiles.
            ACT_FRAC_NUM, ACT_FRAC_DEN = 13, 20  # ~0.65

            def squares(zt, ncols, col_a, col_v):
                act_cols = (ncols * ACT_FRAC_NUM // ACT_FRAC_DEN) // 128 * 128
                sqa = sqp.tile([128, act_cols], mybir.dt.float16, tag="sqa")
                nc.scalar.activation(
                    sqa[:],
                    zt[:, :act_cols],
                    mybir.ActivationFunctionType.Square,
                    accum_out=s_all[:, col_a : col_a + 1],
                )
                sqv = sqp.tile([128, ncols - act_cols], mybir.dt.float16, tag="sqv")
                nc.vector.tensor_mul(sqv[:], zt[:, act_cols:], zt[:, act_cols:])
                nc.vector.reduce_sum(
                    s_all[:, col_v : col_v + 1], sqv[:], axis=mybir.AxisListType.X
                )

            def mm(zt, f_local, b, f_global):
                # start=True clears has_written for the WHOLE bank, so it may
                # only be set on the very first matmul touching this bank;
                # later slices overwrite-on-first-touch via the per-element
                # has_written bits.
                nc.tensor.matmul(
                    psum[:, f_global * NUM_LABELS : (f_global + 1) * NUM_LABELS],
                    zt[:, f_local * 128 : (f_local + 1) * 128],
                    oh_all[:, b * NUM_LABELS : (b + 1) * NUM_LABELS],
                    start=(b == 0 and f_global == 0),
                    stop=(b == N_BTILES - 1 and f_global == N_FCHUNK - 1),
                    skip_group_check=True,
                )

            for b in range(N_BTILES - 1):
                zt = z_tiles[b]
                squares(zt, CH, 2 * b, 2 * b + 1)
                for f in range(N_FCHUNK):
                    mm(zt, f, b, f)

            # Split last tile: half A fully processed while half B transfers.
            HALF_OUT = N_FCHUNK * NUM_LABELS // 2
            out_sb = small.tile([128, N_FCHUNK * NUM_LABELS], mybir.dt.float32)

            squares(z7a, CHH, 14, 15)
            for f in range(N_FCHUNK // 2):
                mm(z7a, f, b7, f)
            # psum cols 0:200 (f 0..19) are final once z7a's matmuls ran;
            # copy + store them while z7b is still transferring/computing.
            nc.vector.tensor_copy(out_sb[:, :HALF_OUT], psum[:, :HALF_OUT])
            nc.sync.dma_start(sums_out[:, :HALF_OUT], out_sb[:, :HALF_OUT])

            squares(z7b, CHH, 16, 17)
            for f in range(N_FCHUNK // 2):
                mm(z7b, f, b7, f + N_FCHUNK // 2)
            nc.vector.tensor_copy(out_sb[:, HALF_OUT:], psum[:, HALF_OUT:])
            nc.sync.dma_start(sums_out[:, HALF_OUT:], out_sb[:, HALF_OUT:])
            # snorm on the scalar ring: issues right after the last
            # accumulator read, in parallel with the sums store.
            nc.scalar.dma_start(snorm_out[:], s_all[:])

    nc.compile()
    return nc


def _get_nc():
    if "nc" not in _CACHE:
        _CACHE["nc"] = _build_nc()
    return _CACHE["nc"]


def kernel(z, labels):
    global LAST_RESULT
    from concourse.bass_utils import run_bass_kernel_spmd

    z = np.asarray(z)
    labels = np.asarray(labels).astype(np.int64)
    assert z.shape == (B_FULL, C, H), z.shape
    z2 = np.nan_to_num(z.reshape(B_FULL, CH)).astype(np.float16)

    onehot = np.zeros((B_FULL, NUM_LABELS), np.float16)
    onehot[np.arange(B_FULL), labels] = 1.0

    in_maps = []
    for c in range(N_CORES):
        zl = z2[c * B_LOCAL : (c + 1) * B_LOCAL]
        oh = (
            onehot[c * B_LOCAL : (c + 1) * B_LOCAL]
            .reshape(N_BTILES, 128, NUM_LABELS)
            .transpose(1, 0, 2)
            .reshape(128, N_BTILES * NUM_LABELS)
        )
        in_maps.append(
            {
                "z": np.ascontiguousarray(zl),
                "onehot": np.ascontiguousarray(oh),
            }
        )

    nc = _get_nc()
    res = run_bass_kernel_spmd(nc, in_maps, core_ids=list(range(N_CORES)))
    LAST_RESULT = res

    # Host gather/unshard: K x CH reduction in float64.
    counts = np.bincount(labels, minlength=NUM_LABELS).astype(np.float64)
    sums = np.zeros((NUM_LABELS, CH), np.float64)
    S2 = np.zeros(NUM_LABELS, np.float64)
    for c in range(N_CORES):
        r = res.results[c]
        arr = np.asarray(r["sums"]).reshape(128, N_FCHUNK, NUM_LABELS)
        sums += arr.transpose(2, 1, 0).reshape(NUM_LABELS, CH)
        sn = np.asarray(r["snorm"]).astype(np.float64)  # [128, 18]
        s_pb = np.empty((128, N_BTILES))
        s_pb[:, : N_BTILES - 1] = (
            sn[:, 0 : 2 * (N_BTILES - 1) : 2] + sn[:, 1 : 2 * (N_BTILES - 1) : 2]
        )
        s_pb[:, N_BTILES - 1] = sn[:, 14:18].sum(axis=1)
        s_flat = s_pb.T.reshape(-1)  # b-major
        lab_loc = labels[c * B_LOCAL : (c + 1) * B_LOCAL]
        S2 += np.bincount(lab_loc, weights=s_flat, minlength=NUM_LABELS)

    c_safe = np.maximum(counts, 1.0)
    sse = S2 - (sums * sums).sum(axis=1) / c_safe + counts * CH * (EPS * EPS)
    mse = sse / (c_safe * CH)
    loss = np.where(counts > 0, mse, 0.0).sum()
    return np.float32(loss)
